# revision 12
# baseline (speedup 1.0000x reference)
"""CP-factorized embedding lookup on 8 TRN2 NeuronCores.

Reference computes full[a,b,c,d,e,f] = sum_r U0[a,r]*...*U5[f,r], reshapes to a
(50000, 512) table, and gathers rows by x. We never materialize the table:

  out[n, e] = sum_r (U0[a_n,r]*U1[b_n,r]*U2[c_n,r]) * (U3[d,r]*U4[e2,r]*U5[f,r])
            = sum_r V[n, r] * W[e, r]

with v = 1000a + 25b + c and e = 64d + 8e2 + f.

Per core (1024 indices, data-parallel over the 8192 total):
  1. x ships as uint16 (lossless: v < 50000 < 65536) and is broadcast across
     115 partitions (50+40+25 stacked factor rows) by two column-half DMAs,
     one per HWDGE ring, so the index chain starts on half 0 while half 1 is
     still in flight. Per half, a 4-op int16 DVE chain in 2x perf mode:
       s1  = rint((v + b1) * R1)          (f32->i16 cast rounds to nearest)
       s2  = rint((v + b2) * R2)
       tkp = K * s2 - OFF                 (per-partition constants)
       onehot = is_equal(s1, tkp) -> bf16
     Rows 0:50 compare a == p with the padding mask folded in as an affine
     step function: s2 = rint((v + 49999.95) * 1e-5) = (v >= 1), so v == 0
     hits no one-hot row -> zero output row. Rows 50:90 compare
     q25 == 40a - 50 + p, rows 90:115 compare (v-25000) == 25*(q25-1000)-90+p.
  2. gather via one PE matmul per half with block-diag stacked [U0;U1;U2]
     (bf16, cast on Scalar) -> psum[96, 512]; V = product of the three
     32-row blocks in 256-col pieces (scalar copy + two DVE mults each).
  3. W[32, 512] = Khatri-Rao of U3,U4,U5 via two broadcast multiplies on
     GpSimd from host-side-transposed factors, before the chain arrives.
  4. out chunk c: matmul(lhsT=V[:,128c:+128], rhs=W f32r) -> psum[128,512],
     copied to SBUF as bf16 (engine rotated: Scalar/GpSimd/Vector) and DMA'd
     per chunk on alternating rings; the final chunk goes in three small
     pieces to shorten the drain tail. Host upcasts bf16 -> f32.

Zero-data warm-up matmuls run on the PE from early startup through the index
chain so the tensor engine reaches its ramped clock (2x faster matmuls)
before the real gather/output matmuls issue.

All constant operands (decomposition table, transposed U3..U5, block-diag
[U0;U1;U2]) ride aux inputs built host-side by pure rearrangement/zero-
padding -- all arithmetic stays on device.
"""

import numpy as np

import concourse.bass as bass
import concourse.mybir as mybir
import concourse.tile as tile
from concourse import bacc
from concourse.bass_utils import run_bass_kernel_spmd

F32 = mybir.dt.float32
F32R = mybir.dt.float32r
BF16 = mybir.dt.bfloat16
I16 = mybir.dt.int16
U16 = mybir.dt.uint16
ALU = mybir.AluOpType

N_CORES = 8
PER_CORE = 1024           # indices per core (8192 / 8)
HALF = 512                # pipeline granularity (one PSUM bank of columns)
EMB = 512
RANK = 32
KV = 115                  # 50 + 40 + 25 stacked vocab-factor rows
MV = 96                   # 3 * RANK stacked outputs

R1000 = float(np.float32(1.0 / 1000.0))
R25 = float(np.float32(1.0 / 25.0))

# aux layout: [115, 6 + 24] constants; aux2: [115, 96] block-diag factors
CC_OFF = 0      # [115, 6] decomposition constants
U345_OFF = 6    # [32, 24] host-transposed U3;U4;U5 (rows 0:32)
AUX_W = 30

N_WARM = 12     # PE warm-up matmuls before the gathers (p-state ramp)
N_FILL = 2      # PE gap fillers between gathers and output matmuls

# per-chunk psum->sbuf copy engine (chunk index 0..7); GpSimd cannot read
# PSUM, so copies alternate Scalar/Vector.
_COPY_ENG = ["scalar", "vector", "scalar", "vector", "scalar", "vector",
             "scalar", "vector"]
# output DMA pieces per chunk: (row_lo, row_hi, ring). All on the otherwise
# idle Sync ring; the last chunk splits small, with GpSimd's software DGE
# issuing the final slivers in parallel to shorten the drain tail.
_PIECES = {c: [(0, 128, "sync")] for c in range(7)}
_PIECES[7] = [(0, 64, "sync"), (64, 96, "gpsimd"), (96, 128, "gpsimd")]


def _const_table() -> np.ndarray:
    """[115, 6] per-partition constants: b1, R1, b2, R2, K, OFF - row.

    s1 = rint((v+b1)*R1); s2 = rint((v+b2)*R2); hit iff s1 == K*s2 - OFF + p.
    """
    cc = np.zeros((KV, 6), np.float32)
    rows = ((0, 50), (50, 90), (90, 115))
    vals = [
        # s1 = a; s2 = (v >= 1); hit iff a == 1000*s2 - 1000 + p
        (-499.5, R1000, 49999.95, 1e-5, 1000.0, 1000.0),
        # s1 = q25; s2 = a; hit iff q25 == 40a - 50 + p  (p abs. row 50..89)
        (-12.0, R25, -499.5, R1000, 40.0, 50.0),
        # s1 = v-25000; s2 = q25-1000; hit iff s1 == 25*s2 - 90 + p
        (-25000.0, 1.0, -25012.0, R25, 25.0, 90.0),
    ]
    for (lo, hi), v6 in zip(rows, vals):
        cc[lo:hi, 0:6] = np.float32(v6)
    cc[:, 5] -= np.arange(KV, dtype=np.float32)
    return cc


def _aux_table(us: list[np.ndarray]) -> tuple[np.ndarray, np.ndarray]:
    aux = np.zeros((KV, AUX_W), np.float32)
    aux[:, CC_OFF:CC_OFF + 6] = _const_table()
    # host-side transpose (pure layout): u345t[r, j] = U{3,4,5}[j, r]
    aux[0:RANK, U345_OFF:U345_OFF + 8] = us[3].T
    aux[0:RANK, U345_OFF + 8:U345_OFF + 16] = us[4].T
    aux[0:RANK, U345_OFF + 16:U345_OFF + 24] = us[5].T
    aux2 = np.zeros((KV, MV), np.float32)
    aux2[0:50, 0:32] = us[0]
    aux2[50:90, 32:64] = us[1]
    aux2[90:115, 64:96] = us[2]
    return aux, aux2


def build():
    nc = bacc.Bacc("TRN2", target_bir_lowering=False, debug=False)

    x = nc.dram_tensor("x", [PER_CORE], U16, kind="ExternalInput")
    aux_d = nc.dram_tensor("aux", [KV, AUX_W], F32, kind="ExternalInput")
    aux2_d = nc.dram_tensor("aux2", [KV, MV], F32, kind="ExternalInput")
    out = nc.dram_tensor("out", [PER_CORE, EMB], BF16, kind="ExternalOutput")

    with tile.TileContext(nc) as tc:
        with (
            tc.tile_pool(name="const", bufs=1) as cpool,
            tc.tile_pool(name="work", bufs=3) as wpool,
            tc.tile_pool(name="vpsum", bufs=2, space="PSUM") as ppool,
            tc.tile_pool(name="osb", bufs=3) as opool,
            tc.tile_pool(name="opsum", bufs=4, space="PSUM") as oppool,
            tc.tile_pool(name="dpsum", bufs=1, space="PSUM") as dpool,
        ):
            # ---- input DMAs. The uint16 index broadcast is split into
            # column halves, one per HWDGE ring, so chain half 0 starts
            # while half 1 is still in flight. aux constants fill the gaps.
            xrep = cpool.tile([KV, PER_CORE], U16)
            nc.sync.dma_start(
                out=xrep[:, 0:HALF],
                in_=x[0:HALF].unsqueeze(0).partition_broadcast(KV),
            )
            aux = cpool.tile([KV, AUX_W], F32)
            nc.scalar.dma_start(out=aux[:], in_=aux_d[:])
            nc.scalar.dma_start(
                out=xrep[:, HALF:PER_CORE],
                in_=x[HALF:PER_CORE].unsqueeze(0).partition_broadcast(KV),
            )
            aux2 = cpool.tile([KV, MV], F32)
            nc.sync.dma_start(out=aux2[:], in_=aux2_d[:])
            cc = aux[:, CC_OFF:CC_OFF + 6]
            u345t = aux[0:RANK, U345_OFF:U345_OFF + 24]

            # ---- PE warm-up on zero data from early startup: shared lhsT,
            # results discarded. Keeps the tensor engine continuously busy
            # through the index chain so real matmuls run at ramped clock.
            warm = cpool.tile([KV, HALF], BF16)
            nc.gpsimd.memset(warm[:], 0.0)
            pd = dpool.tile([MV, HALF], F32)
            for _ in range(N_WARM):
                nc.tensor.matmul(
                    pd[:], lhsT=warm[:, 0:MV], rhs=warm[:], start=True,
                    stop=True,
                )

            # ---- W[r, e] = U3[d,r] * U4[e2,r] * U5[f,r], e = 64d + 8e2 + f
            # on GpSimd, done before the index chain claims the DVE.
            t45 = cpool.tile([RANK, 64], F32)
            nc.gpsimd.tensor_tensor(
                out=t45[:].rearrange("r (e f) -> r e f", e=8),
                in0=u345t[:, 8:16].unsqueeze(2).broadcast_to([RANK, 8, 8]),
                in1=u345t[:, 16:24].unsqueeze(1).broadcast_to([RANK, 8, 8]),
                op=ALU.mult,
            )
            wt = cpool.tile([RANK, EMB], F32R)
            nc.gpsimd.tensor_tensor(
                out=wt[:].rearrange("r (d ef) -> r d ef", d=8),
                in0=u345t[:, 0:8].unsqueeze(2).broadcast_to([RANK, 8, 64]),
                in1=t45[:].unsqueeze(1).broadcast_to([RANK, 8, 64]),
                op=ALU.mult,
            )

            # ---- block-diag [U0;U1;U2] bf16 cast on Scalar (keeps GpSimd
            # and Scalar clear of the DVE chain below)
            ublk = cpool.tile([KV, MV], BF16)
            nc.scalar.copy(out=ublk[:], in_=aux2[:])

            # ---- per-half: 4-op digit chain (int16 2x DVE mode) + gather
            pvs = []
            onehot = cpool.tile([KV, PER_CORE], BF16)
            for h in range(2):
                hs = slice(h * HALF, (h + 1) * HALF)
                s1 = cpool.tile([KV, HALF], I16, name=f"s1_{h}")
                nc.vector.tensor_scalar(
                    out=s1[:], in0=xrep[:, hs], scalar1=cc[:, 0:1],
                    scalar2=cc[:, 1:2], op0=ALU.add, op1=ALU.mult,
                )
                s2 = cpool.tile([KV, HALF], I16, name=f"s2_{h}")
                nc.vector.tensor_scalar(
                    out=s2[:], in0=xrep[:, hs], scalar1=cc[:, 2:3],
                    scalar2=cc[:, 3:4], op0=ALU.add, op1=ALU.mult,
                )
                tkp = cpool.tile([KV, HALF], I16, name=f"tkp_{h}")
                nc.vector.tensor_scalar(
                    out=tkp[:], in0=s2[:], scalar1=cc[:, 4:5],
                    scalar2=cc[:, 5:6], op0=ALU.mult, op1=ALU.subtract,
                )
                nc.vector.tensor_tensor(
                    out=onehot[:, hs], in0=s1[:], in1=tkp[:], op=ALU.is_equal
                )
                pv = ppool.tile([MV, HALF], F32, name=f"pv_{h}", tag="pv")
                nc.tensor.matmul(
                    pv[:], lhsT=ublk[:], rhs=onehot[:, hs],
                    start=True, stop=True,
                )
                pvs.append(pv)

            # PE gap fillers (real rhs so they schedule after chain h0;
            # results discarded) to hold the p-state while V pieces build.
            for _ in range(N_FILL):
                nc.tensor.matmul(
                    pd[:], lhsT=ublk[:], rhs=onehot[:, 0:HALF], start=True,
                    stop=True,
                )

            # V = pv[0:32] * pv[32:64] * pv[64:96], in 256-col pieces, both
            # halves created first so the Vector engine's program order runs
            # all products before any staging copies. DVE may read only one
            # PSUM operand per op: stage block 0 via the Scalar engine.
            QC = HALF // 2
            vths = []
            for h in range(2):
                pv = pvs[h]
                vth = cpool.tile([RANK, HALF], F32R, name=f"vt_{h}")
                for q in range(2):
                    qs = slice(q * QC, (q + 1) * QC)
                    s0 = wpool.tile([RANK, QC], F32, name=f"s0_{h}{q}",
                                    tag="s0")
                    nc.scalar.copy(out=s0[:], in_=pv[0:32, qs])
                    v01 = wpool.tile([RANK, QC], F32, name=f"v01_{h}{q}",
                                     tag="v01")
                    nc.vector.tensor_tensor(
                        out=v01[:], in0=s0[:], in1=pv[32:64, qs], op=ALU.mult
                    )
                    nc.vector.tensor_tensor(
                        out=vth[:, qs], in0=v01[:], in1=pv[64:96, qs],
                        op=ALU.mult,
                    )
                vths.append(vth)

            for c in range(8):                         # chunk index 0..7
                vth = vths[c // 4]
                k = c % 4
                po = oppool.tile([128, EMB], F32, name=f"po_{c}", tag="po")
                nc.tensor.matmul(
                    po[:], lhsT=vth[:, k * 128:(k + 1) * 128], rhs=wt[:],
                    start=True, stop=True,
                )
                osb = opool.tile([128, EMB], BF16, name=f"osb_{c}", tag="osb")
                if _COPY_ENG[c] == "scalar":
                    nc.scalar.copy(out=osb[:], in_=po[:])
                else:
                    nc.vector.tensor_copy(out=osb[:], in_=po[:])

                r0 = c * 128
                for (lo, hi, ring) in _PIECES[c]:
                    eng = {"sync": nc.sync, "scalar": nc.scalar,
                           "gpsimd": nc.gpsimd}[ring]
                    eng.dma_start(
                        out=out[r0 + lo:r0 + hi, :],
                        in_=osb[lo:hi, :],
                    )

    nc.compile()
    return nc


_CACHE: dict = {}


def _get_nc():
    if "nc" not in _CACHE:
        _CACHE["nc"] = build()
    return _CACHE["nc"]


def run(inputs, **spmd_kwargs):
    nc = _get_nc()
    x = np.ascontiguousarray(inputs["x"].reshape(-1)).astype(np.uint16)
    us = [
        np.ascontiguousarray(inputs[f"U{j}"], dtype=np.float32) for j in range(6)
    ]
    aux, aux2 = _aux_table(us)
    in_maps = []
    for i in range(N_CORES):
        in_maps.append(
            {"x": x[i * PER_CORE:(i + 1) * PER_CORE], "aux": aux, "aux2": aux2}
        )
    res = run_bass_kernel_spmd(
        nc, in_maps, core_ids=list(range(N_CORES)), **spmd_kwargs
    )
    shards = [
        np.asarray(res.results[i]["out"]).astype(np.float32)
        for i in range(N_CORES)
    ]
    full = np.concatenate(shards, axis=0).reshape(4, 2048, EMB)
    return full, res


def kernel(**inputs) -> np.ndarray:
    return run(inputs)[0]


# revision 13
# speedup vs baseline: 1.0069x; 1.0069x over previous
"""CP-factorized embedding lookup on 8 TRN2 NeuronCores.

Reference computes full[a,b,c,d,e,f] = sum_r U0[a,r]*...*U5[f,r], reshapes to a
(50000, 512) table, and gathers rows by x. We never materialize the table:

  out[n, e] = sum_r (U0[a_n,r]*U1[b_n,r]*U2[c_n,r]) * (U3[d,r]*U4[e2,r]*U5[f,r])
            = sum_r V[n, r] * W[e, r]

with v = 1000a + 25b + c and e = 64d + 8e2 + f.

Per core (1024 indices, data-parallel over the 8192 total):
  1. x ships as uint16 (lossless: v < 50000 < 65536) and is broadcast across
     115 partitions (50+40+25 stacked factor rows) by two column-half DMAs,
     one per HWDGE ring, so the index chain starts on half 0 while half 1 is
     still in flight. Per half, a 4-op int16 DVE chain in 2x perf mode:
       s1  = rint((v + b1) * R1)          (f32->i16 cast rounds to nearest)
       s2  = rint((v + b2) * R2)
       tkp = K * s2 - OFF                 (per-partition constants)
       onehot = is_equal(s1, tkp) -> bf16
     Rows 0:50 compare a == p with the padding mask folded in as an affine
     step function: s2 = rint((v + 49999.95) * 1e-5) = (v >= 1), so v == 0
     hits no one-hot row -> zero output row. Rows 50:90 compare
     q25 == 40a - 50 + p, rows 90:115 compare (v-25000) == 25*(q25-1000)-90+p.
  2. gather via one PE matmul per half with block-diag stacked [U0;U1;U2]
     (bf16, cast on Scalar) -> psum[96, 512]; V = product of the three
     32-row blocks in 256-col pieces (scalar copy + two DVE mults each).
  3. W[32, 512] = Khatri-Rao of U3,U4,U5 via two broadcast multiplies on
     GpSimd from host-side-transposed factors, before the chain arrives.
  4. out chunk c: matmul(lhsT=V[:,128c:+128], rhs=W f32r) -> psum[128,512],
     copied to SBUF as bf16 (engine rotated: Scalar/GpSimd/Vector) and DMA'd
     per chunk on alternating rings; the final chunk goes in three small
     pieces to shorten the drain tail. Host upcasts bf16 -> f32.

Zero-data warm-up matmuls run on the PE from early startup through the index
chain so the tensor engine reaches its ramped clock (2x faster matmuls)
before the real gather/output matmuls issue.

All constant operands (decomposition table, transposed U3..U5, block-diag
[U0;U1;U2]) ride aux inputs built host-side by pure rearrangement/zero-
padding -- all arithmetic stays on device.
"""

import numpy as np

import concourse.bass as bass
import concourse.mybir as mybir
import concourse.tile as tile
from concourse import bacc
from concourse.bass_utils import run_bass_kernel_spmd

F32 = mybir.dt.float32
F32R = mybir.dt.float32r
BF16 = mybir.dt.bfloat16
I16 = mybir.dt.int16
U16 = mybir.dt.uint16
ALU = mybir.AluOpType

N_CORES = 8
PER_CORE = 1024           # indices per core (8192 / 8)
HALF = 512                # pipeline granularity (one PSUM bank of columns)
EMB = 512
RANK = 32
KV = 115                  # 50 + 40 + 25 stacked vocab-factor rows
MV = 96                   # 3 * RANK stacked outputs

R1000 = float(np.float32(1.0 / 1000.0))
R25 = float(np.float32(1.0 / 25.0))

# aux layout: [115, 6 + 24] constants; aux2: [115, 96] block-diag factors
CC_OFF = 0      # [115, 6] decomposition constants
U345_OFF = 6    # [32, 24] host-transposed U3;U4;U5 (rows 0:32)
AUX_W = 30

N_WARM = 12     # PE warm-up matmuls before the gathers (p-state ramp)
N_FILL = 2      # PE gap fillers between gathers and output matmuls

# per-chunk psum->sbuf copy engine (chunk index 0..7); GpSimd cannot read
# PSUM, so copies alternate Scalar/Vector.
_COPY_ENG = ["scalar", "vector", "scalar", "vector", "scalar", "vector",
             "scalar", "vector"]
# output DMA pieces per chunk: (row_lo, row_hi, ring). All on the otherwise
# idle Sync ring; the last chunk splits small, with GpSimd's software DGE
# issuing the final slivers in parallel to shorten the drain tail.
_PIECES = {c: [(0, 128, "sync")] for c in range(7)}
_PIECES[7] = [(0, 64, "sync"), (64, 96, "gpsimd"), (96, 128, "gpsimd")]


def _const_table() -> np.ndarray:
    """[115, 6] per-partition constants: b1, R1, b2, R2, K, OFF - row.

    s1 = rint((v+b1)*R1); s2 = rint((v+b2)*R2); hit iff s1 == K*s2 - OFF + p.
    """
    cc = np.zeros((KV, 6), np.float32)
    rows = ((0, 50), (50, 90), (90, 115))
    vals = [
        # s1 = a; s2 = (v >= 1); hit iff a == 1000*s2 - 1000 + p
        (-499.5, R1000, 49999.95, 1e-5, 1000.0, 1000.0),
        # s1 = q25; s2 = a; hit iff q25 == 40a - 50 + p  (p abs. row 50..89)
        (-12.0, R25, -499.5, R1000, 40.0, 50.0),
        # s1 = v-25000; s2 = q25-1000; hit iff s1 == 25*s2 - 90 + p
        (-25000.0, 1.0, -25012.0, R25, 25.0, 90.0),
    ]
    for (lo, hi), v6 in zip(rows, vals):
        cc[lo:hi, 0:6] = np.float32(v6)
    cc[:, 5] -= np.arange(KV, dtype=np.float32)
    return cc


def _aux_table(us: list[np.ndarray]) -> tuple[np.ndarray, np.ndarray]:
    aux = np.zeros((KV, AUX_W), np.float32)
    aux[:, CC_OFF:CC_OFF + 6] = _const_table()
    # host-side transpose (pure layout): u345t[r, j] = U{3,4,5}[j, r]
    aux[0:RANK, U345_OFF:U345_OFF + 8] = us[3].T
    aux[0:RANK, U345_OFF + 8:U345_OFF + 16] = us[4].T
    aux[0:RANK, U345_OFF + 16:U345_OFF + 24] = us[5].T
    aux2 = np.zeros((KV, MV), np.float32)
    aux2[0:50, 0:32] = us[0]
    aux2[50:90, 32:64] = us[1]
    aux2[90:115, 64:96] = us[2]
    return aux, aux2


def build():
    nc = bacc.Bacc("TRN2", target_bir_lowering=False, debug=False)

    x = nc.dram_tensor("x", [PER_CORE], U16, kind="ExternalInput")
    aux_d = nc.dram_tensor("aux", [KV, AUX_W], F32, kind="ExternalInput")
    aux2_d = nc.dram_tensor("aux2", [KV, MV], F32, kind="ExternalInput")
    out = nc.dram_tensor("out", [PER_CORE, EMB], BF16, kind="ExternalOutput")

    with tile.TileContext(nc) as tc:
        with (
            tc.tile_pool(name="const", bufs=1) as cpool,
            tc.tile_pool(name="work", bufs=3) as wpool,
            tc.tile_pool(name="vpsum", bufs=2, space="PSUM") as ppool,
            tc.tile_pool(name="osb", bufs=3) as opool,
            tc.tile_pool(name="opsum", bufs=4, space="PSUM") as oppool,
            tc.tile_pool(name="dpsum", bufs=1, space="PSUM") as dpool,
        ):
            # ---- input DMAs. The uint16 index broadcast is split into
            # column halves, one per HWDGE ring, so chain half 0 starts
            # while half 1 is still in flight. aux constants fill the gaps.
            xrep = cpool.tile([KV, PER_CORE], U16)
            nc.sync.dma_start(
                out=xrep[:, 0:HALF],
                in_=x[0:HALF].unsqueeze(0).partition_broadcast(KV),
            )
            aux = cpool.tile([KV, AUX_W], F32)
            nc.scalar.dma_start(out=aux[:], in_=aux_d[:])
            nc.scalar.dma_start(
                out=xrep[:, HALF:PER_CORE],
                in_=x[HALF:PER_CORE].unsqueeze(0).partition_broadcast(KV),
            )
            aux2 = cpool.tile([KV, MV], F32)
            nc.sync.dma_start(out=aux2[:], in_=aux2_d[:])
            cc = aux[:, CC_OFF:CC_OFF + 6]
            u345t = aux[0:RANK, U345_OFF:U345_OFF + 24]

            # ---- PE warm-up on zero data from early startup: shared lhsT,
            # results discarded. Keeps the tensor engine continuously busy
            # through the index chain so real matmuls run at ramped clock.
            warm = cpool.tile([KV, HALF], BF16)
            nc.gpsimd.memset(warm[:], 0.0)
            pd = dpool.tile([MV, HALF], F32)
            for _ in range(N_WARM):
                nc.tensor.matmul(
                    pd[:], lhsT=warm[:, 0:MV], rhs=warm[:], start=True,
                    stop=True,
                )

            # ---- W[r, e] = U3[d,r] * U4[e2,r] * U5[f,r], e = 64d + 8e2 + f
            # on GpSimd, done before the index chain claims the DVE.
            t45 = cpool.tile([RANK, 64], F32)
            nc.gpsimd.tensor_tensor(
                out=t45[:].rearrange("r (e f) -> r e f", e=8),
                in0=u345t[:, 8:16].unsqueeze(2).broadcast_to([RANK, 8, 8]),
                in1=u345t[:, 16:24].unsqueeze(1).broadcast_to([RANK, 8, 8]),
                op=ALU.mult,
            )
            wt = cpool.tile([RANK, EMB], F32R)
            nc.gpsimd.tensor_tensor(
                out=wt[:].rearrange("r (d ef) -> r d ef", d=8),
                in0=u345t[:, 0:8].unsqueeze(2).broadcast_to([RANK, 8, 64]),
                in1=t45[:].unsqueeze(1).broadcast_to([RANK, 8, 64]),
                op=ALU.mult,
            )

            # ---- block-diag [U0;U1;U2] bf16 cast on Scalar (keeps GpSimd
            # and Scalar clear of the DVE chain below)
            ublk = cpool.tile([KV, MV], BF16)
            nc.scalar.copy(out=ublk[:], in_=aux2[:])

            # ---- per-half: 4-op digit chain (int16 2x DVE mode) + gather
            pvs = []
            onehot = cpool.tile([KV, PER_CORE], BF16)
            for h in range(2):
                hs = slice(h * HALF, (h + 1) * HALF)
                s1 = cpool.tile([KV, HALF], I16, name=f"s1_{h}")
                nc.vector.tensor_scalar(
                    out=s1[:], in0=xrep[:, hs], scalar1=cc[:, 0:1],
                    scalar2=cc[:, 1:2], op0=ALU.add, op1=ALU.mult,
                )
                s2 = cpool.tile([KV, HALF], I16, name=f"s2_{h}")
                nc.vector.tensor_scalar(
                    out=s2[:], in0=xrep[:, hs], scalar1=cc[:, 2:3],
                    scalar2=cc[:, 3:4], op0=ALU.add, op1=ALU.mult,
                )
                tkp = cpool.tile([KV, HALF], I16, name=f"tkp_{h}")
                nc.vector.tensor_scalar(
                    out=tkp[:], in0=s2[:], scalar1=cc[:, 4:5],
                    scalar2=cc[:, 5:6], op0=ALU.mult, op1=ALU.subtract,
                )
                nc.vector.tensor_tensor(
                    out=onehot[:, hs], in0=s1[:], in1=tkp[:], op=ALU.is_equal
                )
                pv = ppool.tile([MV, HALF], F32, name=f"pv_{h}", tag="pv")
                nc.tensor.matmul(
                    pv[:], lhsT=ublk[:], rhs=onehot[:, hs],
                    start=True, stop=True,
                )
                pvs.append(pv)

            # PE gap fillers (real rhs so they schedule after chain h0;
            # results discarded) to hold the p-state while V pieces build.
            for _ in range(N_FILL):
                nc.tensor.matmul(
                    pd[:], lhsT=ublk[:], rhs=onehot[:, 0:HALF], start=True,
                    stop=True,
                )

            # V = pv[0:32] * pv[32:64] * pv[64:96], in 256-col pieces, both
            # halves created first so the Vector engine's program order runs
            # all products before any staging copies. DVE may read only one
            # PSUM operand per op: stage block 0 via the Scalar engine.
            QC = HALF // 2
            vths = []
            with tc.high_priority():
                for h in range(2):
                    pv = pvs[h]
                    vth = cpool.tile([RANK, HALF], F32R, name=f"vt_{h}")
                    for q in range(2):
                        qs = slice(q * QC, (q + 1) * QC)
                        # dedicated (unpooled) tiles: pooled buffers add
                        # conservative reuse waits that stall the pipeline
                        s0 = cpool.tile([RANK, QC], F32, name=f"s0_{h}{q}")
                        nc.scalar.copy(out=s0[:], in_=pv[0:32, qs])
                        v01 = cpool.tile([RANK, QC], F32, name=f"v01_{h}{q}")
                        nc.vector.tensor_tensor(
                            out=v01[:], in0=s0[:], in1=pv[32:64, qs],
                            op=ALU.mult,
                        )
                        nc.vector.tensor_tensor(
                            out=vth[:, qs], in0=v01[:], in1=pv[64:96, qs],
                            op=ALU.mult,
                        )
                    vths.append(vth)

            for c in range(8):                         # chunk index 0..7
                vth = vths[c // 4]
                k = c % 4
                po = oppool.tile([128, EMB], F32, name=f"po_{c}", tag="po")
                nc.tensor.matmul(
                    po[:], lhsT=vth[:, k * 128:(k + 1) * 128], rhs=wt[:],
                    start=True, stop=True,
                )
                osb = opool.tile([128, EMB], BF16, name=f"osb_{c}", tag="osb")
                if _COPY_ENG[c] == "scalar":
                    nc.scalar.copy(out=osb[:], in_=po[:])
                else:
                    nc.vector.tensor_copy(out=osb[:], in_=po[:])

                r0 = c * 128
                for (lo, hi, ring) in _PIECES[c]:
                    eng = {"sync": nc.sync, "scalar": nc.scalar,
                           "gpsimd": nc.gpsimd}[ring]
                    eng.dma_start(
                        out=out[r0 + lo:r0 + hi, :],
                        in_=osb[lo:hi, :],
                    )

    nc.compile()
    return nc


_CACHE: dict = {}


def _get_nc():
    if "nc" not in _CACHE:
        _CACHE["nc"] = build()
    return _CACHE["nc"]


def run(inputs, **spmd_kwargs):
    nc = _get_nc()
    x = np.ascontiguousarray(inputs["x"].reshape(-1)).astype(np.uint16)
    us = [
        np.ascontiguousarray(inputs[f"U{j}"], dtype=np.float32) for j in range(6)
    ]
    aux, aux2 = _aux_table(us)
    in_maps = []
    for i in range(N_CORES):
        in_maps.append(
            {"x": x[i * PER_CORE:(i + 1) * PER_CORE], "aux": aux, "aux2": aux2}
        )
    res = run_bass_kernel_spmd(
        nc, in_maps, core_ids=list(range(N_CORES)), **spmd_kwargs
    )
    shards = [
        np.asarray(res.results[i]["out"]).astype(np.float32)
        for i in range(N_CORES)
    ]
    full = np.concatenate(shards, axis=0).reshape(4, 2048, EMB)
    return full, res


def kernel(**inputs) -> np.ndarray:
    return run(inputs)[0]


# revision 17
# speedup vs baseline: 1.0904x; 1.0830x over previous
"""CP-factorized embedding lookup on 8 TRN2 NeuronCores.

Reference computes full[a,b,c,d,e,f] = sum_r U0[a,r]*...*U5[f,r], reshapes to a
(50000, 512) table, and gathers rows by x. We never materialize the table:

  out[n, e] = sum_r (U0[a_n,r]*U1[b_n,r]*U2[c_n,r]) * (U3[d,r]*U4[e2,r]*U5[f,r])
            = sum_r V[n, r] * W[e, r]

with v = 1000a + 25b + c and e = 64d + 8e2 + f.

Per core (1024 indices, data-parallel over the 8192 total):
  1. x ships as uint16 (lossless: v < 50000 < 65536) and is broadcast across
     115 partitions (50+40+25 stacked factor rows) by two column-half DMAs,
     one per HWDGE ring, so the index chain starts on half 0 while half 1 is
     still in flight. Per half, a 4-op int16 DVE chain in 2x perf mode:
       s1  = rint((v + b1) * R1)          (f32->i16 cast rounds to nearest)
       s2  = rint((v + b2) * R2)
       tkp = K * s2 - OFF                 (per-partition constants)
       onehot = is_equal(s1, tkp) -> bf16
     Rows 0:50 compare a == p with the padding mask folded in as an affine
     step function: s2 = rint((v + 49999.95) * 1e-5) = (v >= 1), so v == 0
     hits no one-hot row -> zero output row. Rows 50:90 compare
     q25 == 40a - 50 + p, rows 90:115 compare (v-25000) == 25*(q25-1000)-90+p.
  2. gather via one PE matmul per half with block-diag stacked [U0;U1;U2]
     (bf16, cast on Scalar) -> psum[96, 512]; V = product of the three
     32-row blocks in 256-col pieces (scalar copy + two DVE mults each).
  3. W[32, 512] = Khatri-Rao of U3,U4,U5 via two broadcast multiplies on
     GpSimd from host-side-transposed factors, before the chain arrives.
  4. out chunk c: matmul(lhsT=V[:,128c:+128], rhs=W f32r) -> psum[128,512],
     copied to SBUF as bf16 (engine rotated: Scalar/GpSimd/Vector) and DMA'd
     per chunk on alternating rings; the final chunk goes in three small
     pieces to shorten the drain tail. Host upcasts bf16 -> f32.

Zero-data warm-up matmuls run on the PE from early startup through the index
chain so the tensor engine reaches its ramped clock (2x faster matmuls)
before the real gather/output matmuls issue.

All constant operands (decomposition table, transposed U3..U5, block-diag
[U0;U1;U2]) ride aux inputs built host-side by pure rearrangement/zero-
padding -- all arithmetic stays on device.
"""

import numpy as np

import concourse.bass as bass
import concourse.mybir as mybir
import concourse.tile as tile
from concourse import bacc
from concourse.bass_utils import run_bass_kernel_spmd

F32 = mybir.dt.float32
F32R = mybir.dt.float32r
BF16 = mybir.dt.bfloat16
I16 = mybir.dt.int16
U16 = mybir.dt.uint16
ALU = mybir.AluOpType

N_CORES = 8
PER_CORE = 1024           # indices per core (8192 / 8)
HALF = 512                # pipeline granularity (one PSUM bank of columns)
EMB = 512
RANK = 32
KV = 115                  # 50 + 40 + 25 stacked vocab-factor rows
MV = 96                   # 3 * RANK stacked outputs

R1000 = float(np.float32(1.0 / 1000.0))
R25 = float(np.float32(1.0 / 25.0))

# aux layout: [115, 6 + 24] constants; aux2: [115, 96] block-diag factors
CC_OFF = 0      # [115, 6] decomposition constants
U345_OFF = 6    # [32, 24] host-transposed U3;U4;U5 (rows 0:32)
AUX_W = 30

N_WARM = 12     # PE warm-up matmuls before the gathers (p-state ramp)
N_FILL = 3      # PE gap fillers between gathers and output matmuls

# per-chunk psum->sbuf copy engine (chunk index 0..7); GpSimd cannot read
# PSUM, so copies alternate Scalar/Vector.
_COPY_ENG = ["scalar", "vector", "scalar", "vector", "scalar", "vector",
             "scalar", "vector"]
# output DMA pieces per chunk: (row_lo, row_hi, ring). All on the otherwise
# idle Sync ring; the last chunk splits small, with GpSimd's software DGE
# issuing the final slivers in parallel to shorten the drain tail.
_PIECES = {c: [(0, 128, "sync")] for c in range(7)}
_PIECES[7] = [(0, 64, "sync"), (64, 96, "gpsimd"), (96, 128, "gpsimd")]


def _const_table() -> np.ndarray:
    """[115, 6] per-partition constants: b1, R1, b2, R2, K, OFF - row.

    s1 = rint((v+b1)*R1); s2 = rint((v+b2)*R2); hit iff s1 == K*s2 - OFF + p.
    """
    cc = np.zeros((KV, 6), np.float32)
    rows = ((0, 50), (50, 90), (90, 115))
    vals = [
        # s1 = a; s2 = (v >= 1); hit iff a == 1000*s2 - 1000 + p
        (-499.5, R1000, 49999.95, 1e-5, 1000.0, 1000.0),
        # s1 = q25; s2 = a; hit iff q25 == 40a - 50 + p  (p abs. row 50..89)
        (-12.0, R25, -499.5, R1000, 40.0, 50.0),
        # s1 = v-25000; s2 = q25-1000; hit iff s1 == 25*s2 - 90 + p
        (-25000.0, 1.0, -25012.0, R25, 25.0, 90.0),
    ]
    for (lo, hi), v6 in zip(rows, vals):
        cc[lo:hi, 0:6] = np.float32(v6)
    cc[:, 5] -= np.arange(KV, dtype=np.float32)
    return cc


def _aux_table(us: list[np.ndarray]) -> tuple[np.ndarray, np.ndarray]:
    aux = np.zeros((KV, AUX_W), np.float32)
    aux[:, CC_OFF:CC_OFF + 6] = _const_table()
    # host-side transpose (pure layout): u345t[r, j] = U{3,4,5}[j, r]
    aux[0:RANK, U345_OFF:U345_OFF + 8] = us[3].T
    aux[0:RANK, U345_OFF + 8:U345_OFF + 16] = us[4].T
    aux[0:RANK, U345_OFF + 16:U345_OFF + 24] = us[5].T
    aux2 = np.zeros((KV, MV), np.float32)
    aux2[0:50, 0:32] = us[0]
    aux2[50:90, 32:64] = us[1]
    aux2[90:115, 64:96] = us[2]
    return aux, aux2


def build():
    nc = bacc.Bacc("TRN2", target_bir_lowering=False, debug=False)

    x = nc.dram_tensor("x", [PER_CORE], U16, kind="ExternalInput")
    aux_d = nc.dram_tensor("aux", [KV, AUX_W], F32, kind="ExternalInput")
    aux2_d = nc.dram_tensor("aux2", [KV, MV], F32, kind="ExternalInput")
    out = nc.dram_tensor("out", [PER_CORE, EMB], BF16, kind="ExternalOutput")

    with tile.TileContext(nc) as tc:
        with (
            tc.tile_pool(name="const", bufs=1) as cpool,
            tc.tile_pool(name="work", bufs=3) as wpool,
            tc.tile_pool(name="vpsum", bufs=2, space="PSUM") as ppool,
            tc.tile_pool(name="osb", bufs=8) as opool,
            tc.tile_pool(name="opsum", bufs=5, space="PSUM") as oppool,
            tc.tile_pool(name="dpsum", bufs=1, space="PSUM") as dpool,
        ):
            # ---- input DMAs. The uint16 index broadcast is split into
            # column halves, one per HWDGE ring, so chain half 0 starts
            # while half 1 is still in flight. aux constants fill the gaps.
            xrep = cpool.tile([KV, PER_CORE], U16)
            nc.sync.dma_start(
                out=xrep[:, 0:HALF],
                in_=x[0:HALF].unsqueeze(0).partition_broadcast(KV),
            )
            aux = cpool.tile([KV, AUX_W], F32)
            nc.scalar.dma_start(out=aux[:], in_=aux_d[:])
            nc.scalar.dma_start(
                out=xrep[:, HALF:PER_CORE],
                in_=x[HALF:PER_CORE].unsqueeze(0).partition_broadcast(KV),
            )
            aux2 = cpool.tile([KV, MV], F32)
            nc.sync.dma_start(out=aux2[:], in_=aux2_d[:])
            cc = aux[:, CC_OFF:CC_OFF + 6]
            u345t = aux[0:RANK, U345_OFF:U345_OFF + 24]

            # ---- PE warm-up on zero data from early startup: shared lhsT,
            # results discarded. Keeps the tensor engine continuously busy
            # through the index chain so real matmuls run at ramped clock.
            warm = cpool.tile([KV, HALF], BF16)
            nc.gpsimd.memset(warm[:], 0.0)
            pd = dpool.tile([MV, HALF], F32)
            for _ in range(N_WARM):
                nc.tensor.matmul(
                    pd[:], lhsT=warm[:, 0:MV], rhs=warm[:], start=True,
                    stop=True,
                )

            # ---- W[r, e] = U3[d,r] * U4[e2,r] * U5[f,r], e = 64d + 8e2 + f
            # on GpSimd, done before the index chain claims the DVE.
            t45 = cpool.tile([RANK, 64], F32)
            nc.gpsimd.tensor_tensor(
                out=t45[:].rearrange("r (e f) -> r e f", e=8),
                in0=u345t[:, 8:16].unsqueeze(2).broadcast_to([RANK, 8, 8]),
                in1=u345t[:, 16:24].unsqueeze(1).broadcast_to([RANK, 8, 8]),
                op=ALU.mult,
            )
            wt = cpool.tile([RANK, EMB], F32R)
            nc.gpsimd.tensor_tensor(
                out=wt[:].rearrange("r (d ef) -> r d ef", d=8),
                in0=u345t[:, 0:8].unsqueeze(2).broadcast_to([RANK, 8, 64]),
                in1=t45[:].unsqueeze(1).broadcast_to([RANK, 8, 64]),
                op=ALU.mult,
            )

            # ---- block-diag [U0;U1;U2] bf16 cast on Scalar (keeps GpSimd
            # and Scalar clear of the DVE chain below)
            ublk = cpool.tile([KV, MV], BF16)
            nc.scalar.copy(out=ublk[:], in_=aux2[:])

            # ---- per-half: 4-op digit chain (int16 2x DVE mode) + gather
            pvs = []
            onehot = cpool.tile([KV, PER_CORE], BF16)
            for h in range(2):
                hs = slice(h * HALF, (h + 1) * HALF)
                s1 = cpool.tile([KV, HALF], I16, name=f"s1_{h}")
                nc.vector.tensor_scalar(
                    out=s1[:], in0=xrep[:, hs], scalar1=cc[:, 0:1],
                    scalar2=cc[:, 1:2], op0=ALU.add, op1=ALU.mult,
                )
                s2 = cpool.tile([KV, HALF], I16, name=f"s2_{h}")
                nc.vector.tensor_scalar(
                    out=s2[:], in0=xrep[:, hs], scalar1=cc[:, 2:3],
                    scalar2=cc[:, 3:4], op0=ALU.add, op1=ALU.mult,
                )
                tkp = cpool.tile([KV, HALF], I16, name=f"tkp_{h}")
                nc.vector.tensor_scalar(
                    out=tkp[:], in0=s2[:], scalar1=cc[:, 4:5],
                    scalar2=cc[:, 5:6], op0=ALU.mult, op1=ALU.subtract,
                )
                nc.vector.tensor_tensor(
                    out=onehot[:, hs], in0=s1[:], in1=tkp[:], op=ALU.is_equal
                )
                pv = ppool.tile([MV, HALF], F32, name=f"pv_{h}", tag="pv")
                nc.tensor.matmul(
                    pv[:], lhsT=ublk[:], rhs=onehot[:, hs],
                    start=True, stop=True,
                )
                pvs.append(pv)

            # PE gap fillers (real rhs so they schedule after chain h0;
            # results discarded) to hold the p-state while V pieces build.
            for _ in range(N_FILL):
                nc.tensor.matmul(
                    pd[:], lhsT=ublk[:], rhs=onehot[:, 0:HALF], start=True,
                    stop=True,
                )

            # V = pv[0:32] * pv[32:64] * pv[64:96], in 256-col pieces, both
            # halves created first so the Vector engine's program order runs
            # all products before any staging copies. DVE may read only one
            # PSUM operand per op: stage block 0 via the Scalar engine.
            QC = HALF // 2
            vths = []
            with tc.high_priority():
                for h in range(2):
                    pv = pvs[h]
                    # one full-half scalar staging copy (a single op per half
                    # keeps the compile-time list scheduler from reordering
                    # piece copies behind later-half work)
                    s0 = cpool.tile([RANK, HALF], F32, name=f"s0_{h}")
                    nc.scalar.copy(out=s0[:], in_=pv[0:32, :])
                    # products in 256-col pieces; the second multiply runs
                    # in place so each piece is a WAW-ordered DVE chain
                    vth = cpool.tile([RANK, HALF], F32R, name=f"vt_{h}")
                    for q in range(2):
                        qs = slice(q * QC, (q + 1) * QC)
                        nc.vector.tensor_tensor(
                            out=vth[:, qs], in0=s0[:, qs], in1=pv[32:64, qs],
                            op=ALU.mult,
                        )
                        nc.vector.tensor_tensor(
                            out=vth[:, qs], in0=vth[:, qs], in1=pv[64:96, qs],
                            op=ALU.mult,
                        )
                    vths.append(vth)

            for c in range(8):                         # chunk index 0..7
                vth = vths[c // 4]
                k = c % 4
                po = oppool.tile([128, EMB], F32, name=f"po_{c}", tag="po")
                nc.tensor.matmul(
                    po[:], lhsT=vth[:, k * 128:(k + 1) * 128], rhs=wt[:],
                    start=True, stop=True,
                )
                osb = opool.tile([128, EMB], BF16, name=f"osb_{c}", tag="osb")
                if _COPY_ENG[c] == "scalar":
                    nc.scalar.copy(out=osb[:], in_=po[:])
                else:
                    nc.vector.tensor_copy(out=osb[:], in_=po[:])

                r0 = c * 128
                for (lo, hi, ring) in _PIECES[c]:
                    eng = {"sync": nc.sync, "scalar": nc.scalar,
                           "gpsimd": nc.gpsimd}[ring]
                    eng.dma_start(
                        out=out[r0 + lo:r0 + hi, :],
                        in_=osb[lo:hi, :],
                    )

    nc.compile()
    return nc


_CACHE: dict = {}


def _get_nc():
    if "nc" not in _CACHE:
        _CACHE["nc"] = build()
    return _CACHE["nc"]


def run(inputs, **spmd_kwargs):
    nc = _get_nc()
    x = np.ascontiguousarray(inputs["x"].reshape(-1)).astype(np.uint16)
    us = [
        np.ascontiguousarray(inputs[f"U{j}"], dtype=np.float32) for j in range(6)
    ]
    aux, aux2 = _aux_table(us)
    in_maps = []
    for i in range(N_CORES):
        in_maps.append(
            {"x": x[i * PER_CORE:(i + 1) * PER_CORE], "aux": aux, "aux2": aux2}
        )
    res = run_bass_kernel_spmd(
        nc, in_maps, core_ids=list(range(N_CORES)), **spmd_kwargs
    )
    shards = [
        np.asarray(res.results[i]["out"]).astype(np.float32)
        for i in range(N_CORES)
    ]
    full = np.concatenate(shards, axis=0).reshape(4, 2048, EMB)
    return full, res


def kernel(**inputs) -> np.ndarray:
    return run(inputs)[0]


# revision 21
# speedup vs baseline: 1.1345x; 1.0404x over previous
"""CP-factorized embedding lookup on 8 TRN2 NeuronCores.

Reference computes full[a,b,c,d,e,f] = sum_r U0[a,r]*...*U5[f,r], reshapes to a
(50000, 512) table, and gathers rows by x. We never materialize the table:

  out[n, e] = sum_r (U0[a_n,r]*U1[b_n,r]*U2[c_n,r]) * (U3[d,r]*U4[e2,r]*U5[f,r])
            = sum_r V[n, r] * W[e, r]

with v = 1000a + 25b + c and e = 64d + 8e2 + f.

Per core (1024 indices, data-parallel over the 8192 total):
  1. x ships as uint16 (lossless: v < 50000 < 65536) and is broadcast across
     115 partitions (50+40+25 stacked factor rows) by two column-half DMAs,
     one per HWDGE ring, so the index chain starts on half 0 while half 1 is
     still in flight. Per half, a 4-op int16 DVE chain in 2x perf mode:
       s1  = rint((v + b1) * R1)          (f32->i16 cast rounds to nearest)
       s2  = rint((v + b2) * R2)
       tkp = K * s2 - OFF                 (per-partition constants)
       onehot = is_equal(s1, tkp) -> bf16
     Rows 0:50 compare a == p with the padding mask folded in as an affine
     step function: s2 = rint((v + 49999.95) * 1e-5) = (v >= 1), so v == 0
     hits no one-hot row -> zero output row. Rows 50:90 compare
     q25 == 40a - 50 + p, rows 90:115 compare (v-25000) == 25*(q25-1000)-90+p.
  2. gather via one PE matmul per half with block-diag stacked [U0;U1;U2]
     (bf16, cast on Scalar) -> psum[96, 512]; V = product of the three
     32-row blocks in 256-col pieces (scalar copy + two DVE mults each).
  3. W[32, 512] = Khatri-Rao of U3,U4,U5 via two broadcast multiplies on
     GpSimd from host-side-transposed factors, before the chain arrives.
  4. out chunk c: matmul(lhsT=V[:,128c:+128], rhs=W f32r) -> psum[128,512],
     copied to SBUF as bf16 (engine rotated: Scalar/GpSimd/Vector) and DMA'd
     per chunk on alternating rings; the final chunk goes in three small
     pieces to shorten the drain tail. Host upcasts bf16 -> f32.

Zero-data warm-up matmuls run on the PE from early startup through the index
chain so the tensor engine reaches its ramped clock (2x faster matmuls)
before the real gather/output matmuls issue.

All constant operands (decomposition table, transposed U3..U5, block-diag
[U0;U1;U2]) ride aux inputs built host-side by pure rearrangement/zero-
padding -- all arithmetic stays on device.
"""

import numpy as np

import concourse.bass as bass
import concourse.mybir as mybir
import concourse.tile as tile
from concourse import bacc
from concourse.bass_utils import run_bass_kernel_spmd

F32 = mybir.dt.float32
F32R = mybir.dt.float32r
BF16 = mybir.dt.bfloat16
I16 = mybir.dt.int16
U16 = mybir.dt.uint16
ALU = mybir.AluOpType

N_CORES = 8
PER_CORE = 1024           # indices per core (8192 / 8)
HALF = 512                # pipeline granularity (one PSUM bank of columns)
EMB = 512
RANK = 32
KV = 115                  # 50 + 40 + 25 stacked vocab-factor rows
MV = 96                   # 3 * RANK stacked outputs

R1000 = float(np.float32(1.0 / 1000.0))
R25 = float(np.float32(1.0 / 25.0))

# aux layout: [115, 6 + 24] constants; aux2: [115, 96] block-diag factors
CC_OFF = 0      # [115, 6] decomposition constants
U345_OFF = 6    # [32, 24] host-transposed U3;U4;U5 (rows 0:32)
AUX_W = 30

N_WARM = 14     # PE warm-up matmuls before the gathers (p-state ramp)
N_FILL = 3      # PE gap fillers between gathers and output matmuls

# per-chunk psum->sbuf copy engine (chunk index 0..7); GpSimd cannot read
# PSUM, so copies alternate Scalar/Vector.
_COPY_ENG = ["scalar", "vector", "scalar", "vector", "scalar", "vector",
             "scalar", "vector"]
# output DMA pieces per chunk: (row_lo, row_hi, ring), spread over the Sync
# ring, GpSimd's software DGE, and one on Scalar so no single issuer
# backlogs; the last chunk splits small to shorten the drain tail.
_PIECES = {c: [(0, 128, "sync")] for c in range(7)}
_PIECES[1] = [(0, 128, "gpsimd")]
_PIECES[3] = [(0, 128, "scalar")]
_PIECES[5] = [(0, 128, "gpsimd")]
_PIECES[7] = [(0, 64, "sync"), (64, 96, "gpsimd"), (96, 128, "gpsimd")]


def _const_table() -> np.ndarray:
    """[115, 6] per-partition constants: b1, R1, b2, R2, K, OFF - row.

    s1 = rint((v+b1)*R1); s2 = rint((v+b2)*R2); hit iff s1 == K*s2 - OFF + p.
    """
    cc = np.zeros((KV, 6), np.float32)
    rows = ((0, 50), (50, 90), (90, 115))
    vals = [
        # s1 = a; s2 = (v >= 1); hit iff a == 1000*s2 - 1000 + p
        (-499.5, R1000, 49999.95, 1e-5, 1000.0, 1000.0),
        # s1 = q25; s2 = a; hit iff q25 == 40a - 50 + p  (p abs. row 50..89)
        (-12.0, R25, -499.5, R1000, 40.0, 50.0),
        # s1 = v-25000; s2 = q25-1000; hit iff s1 == 25*s2 - 90 + p
        (-25000.0, 1.0, -25012.0, R25, 25.0, 90.0),
    ]
    for (lo, hi), v6 in zip(rows, vals):
        cc[lo:hi, 0:6] = np.float32(v6)
    cc[:, 5] -= np.arange(KV, dtype=np.float32)
    return cc


def _aux_table(us: list[np.ndarray]) -> tuple[np.ndarray, np.ndarray]:
    aux = np.zeros((KV, AUX_W), np.float32)
    aux[:, CC_OFF:CC_OFF + 6] = _const_table()
    # host-side transpose (pure layout): u345t[r, j] = U{3,4,5}[j, r]
    aux[0:RANK, U345_OFF:U345_OFF + 8] = us[3].T
    aux[0:RANK, U345_OFF + 8:U345_OFF + 16] = us[4].T
    aux[0:RANK, U345_OFF + 16:U345_OFF + 24] = us[5].T
    aux2 = np.zeros((KV, MV), np.float32)
    aux2[0:50, 0:32] = us[0]
    aux2[50:90, 32:64] = us[1]
    aux2[90:115, 64:96] = us[2]
    return aux, aux2


def build():
    nc = bacc.Bacc("TRN2", target_bir_lowering=False, debug=False)

    x = nc.dram_tensor("x", [PER_CORE], U16, kind="ExternalInput")
    aux_d = nc.dram_tensor("aux", [KV, AUX_W], F32, kind="ExternalInput")
    aux2_d = nc.dram_tensor("aux2", [KV, MV], F32, kind="ExternalInput")
    out = nc.dram_tensor("out", [PER_CORE, EMB], BF16, kind="ExternalOutput")

    with tile.TileContext(nc) as tc:
        with (
            tc.tile_pool(name="const", bufs=1) as cpool,
            tc.tile_pool(name="work", bufs=3) as wpool,
            tc.tile_pool(name="vpsum", bufs=2, space="PSUM") as ppool,
            tc.tile_pool(name="osb", bufs=8) as opool,
            tc.tile_pool(name="opsum", bufs=5, space="PSUM") as oppool,
            tc.tile_pool(name="dpsum", bufs=1, space="PSUM") as dpool,
        ):
            # ---- input DMAs. The uint16 index broadcast is split into
            # column halves, one per HWDGE ring, so chain half 0 starts
            # while half 1 is still in flight. aux constants fill the gaps.
            aux = cpool.tile([KV, AUX_W], F32)
            nc.sync.dma_start(out=aux[:], in_=aux_d[:])
            xrep = cpool.tile([KV, PER_CORE], U16)
            nc.sync.dma_start(
                out=xrep[:, 0:HALF],
                in_=x[0:HALF].unsqueeze(0).partition_broadcast(KV),
            )
            nc.scalar.dma_start(
                out=xrep[:, HALF:PER_CORE],
                in_=x[HALF:PER_CORE].unsqueeze(0).partition_broadcast(KV),
            )
            aux2 = cpool.tile([KV, MV], F32)
            nc.scalar.dma_start(out=aux2[:], in_=aux2_d[:])
            cc = aux[:, CC_OFF:CC_OFF + 6]
            u345t = aux[0:RANK, U345_OFF:U345_OFF + 24]

            # ---- PE warm-up on zero data from early startup: shared lhsT,
            # results discarded. Keeps the tensor engine continuously busy
            # through the index chain so real matmuls run at ramped clock.
            warm = cpool.tile([KV, HALF], BF16)
            nc.gpsimd.memset(warm[:], 0.0)
            pd = dpool.tile([MV, HALF], F32)
            for _ in range(N_WARM):
                nc.tensor.matmul(
                    pd[:], lhsT=warm[:, 0:MV], rhs=warm[:], start=True,
                    stop=True,
                )

            # ---- W[r, e] = U3[d,r] * U4[e2,r] * U5[f,r], e = 64d + 8e2 + f
            # on the DVE before the index broadcast lands (aux rides first on
            # the sync ring; concurrent GpSimd work would contend for SBUF
            # ports and slow the chain ops, so GpSimd stays idle here).
            t45 = cpool.tile([RANK, 64], F32)
            nc.vector.tensor_tensor(
                out=t45[:].rearrange("r (e f) -> r e f", e=8),
                in0=u345t[:, 8:16].unsqueeze(2).broadcast_to([RANK, 8, 8]),
                in1=u345t[:, 16:24].unsqueeze(1).broadcast_to([RANK, 8, 8]),
                op=ALU.mult,
            )
            wt = cpool.tile([RANK, EMB], F32R)
            nc.vector.tensor_tensor(
                out=wt[:].rearrange("r (d ef) -> r d ef", d=8),
                in0=u345t[:, 0:8].unsqueeze(2).broadcast_to([RANK, 8, 64]),
                in1=t45[:].unsqueeze(1).broadcast_to([RANK, 8, 64]),
                op=ALU.mult,
            )

            # ---- block-diag [U0;U1;U2] bf16 cast on Scalar (keeps GpSimd
            # and Scalar clear of the DVE chain below)
            ublk = cpool.tile([KV, MV], BF16)
            nc.scalar.copy(out=ublk[:], in_=aux2[:])

            # ---- per-half: 4-op digit chain (int16 2x DVE mode) + gather
            pvs = []
            onehot = cpool.tile([KV, PER_CORE], BF16)
            for h in range(2):
                hs = slice(h * HALF, (h + 1) * HALF)
                s1 = cpool.tile([KV, HALF], I16, name=f"s1_{h}")
                nc.vector.tensor_scalar(
                    out=s1[:], in0=xrep[:, hs], scalar1=cc[:, 0:1],
                    scalar2=cc[:, 1:2], op0=ALU.add, op1=ALU.mult,
                )
                s2 = cpool.tile([KV, HALF], I16, name=f"s2_{h}")
                nc.vector.tensor_scalar(
                    out=s2[:], in0=xrep[:, hs], scalar1=cc[:, 2:3],
                    scalar2=cc[:, 3:4], op0=ALU.add, op1=ALU.mult,
                )
                tkp = cpool.tile([KV, HALF], I16, name=f"tkp_{h}")
                nc.vector.tensor_scalar(
                    out=tkp[:], in0=s2[:], scalar1=cc[:, 4:5],
                    scalar2=cc[:, 5:6], op0=ALU.mult, op1=ALU.subtract,
                )
                nc.vector.tensor_tensor(
                    out=onehot[:, hs], in0=s1[:], in1=tkp[:], op=ALU.is_equal
                )
                pv = ppool.tile([MV, HALF], F32, name=f"pv_{h}", tag="pv")
                nc.tensor.matmul(
                    pv[:], lhsT=ublk[:], rhs=onehot[:, hs],
                    start=True, stop=True,
                )
                pvs.append(pv)

            # PE gap fillers (real rhs so they schedule after chain h0;
            # results discarded) to hold the p-state while V pieces build.
            for _ in range(N_FILL):
                nc.tensor.matmul(
                    pd[:], lhsT=ublk[:], rhs=onehot[:, 0:HALF], start=True,
                    stop=True,
                )

            # V = pv[0:32] * pv[32:64] * pv[64:96], in 256-col pieces, both
            # halves created first so the Vector engine's program order runs
            # all products before any staging copies. DVE may read only one
            # PSUM operand per op: stage block 0 via the Scalar engine.
            QC = HALF // 2
            vths = []
            with tc.high_priority():
                for h in range(2):
                    pv = pvs[h]
                    # one full-half scalar staging copy (a single op per half
                    # keeps the compile-time list scheduler from reordering
                    # piece copies behind later-half work)
                    s0 = cpool.tile([RANK, HALF], F32, name=f"s0_{h}")
                    nc.scalar.copy(out=s0[:], in_=pv[0:32, :])
                    # products in 256-col pieces; the second multiply runs
                    # in place so each piece is a WAW-ordered DVE chain
                    vth = cpool.tile([RANK, HALF], F32R, name=f"vt_{h}")
                    for q in range(2):
                        qs = slice(q * QC, (q + 1) * QC)
                        nc.vector.tensor_tensor(
                            out=vth[:, qs], in0=s0[:, qs], in1=pv[32:64, qs],
                            op=ALU.mult,
                        )
                        nc.vector.tensor_tensor(
                            out=vth[:, qs], in0=vth[:, qs], in1=pv[64:96, qs],
                            op=ALU.mult,
                        )
                    vths.append(vth)

            for c in range(8):                         # chunk index 0..7
                vth = vths[c // 4]
                k = c % 4
                po = oppool.tile([128, EMB], F32, name=f"po_{c}", tag="po")
                nc.tensor.matmul(
                    po[:], lhsT=vth[:, k * 128:(k + 1) * 128], rhs=wt[:],
                    start=True, stop=True,
                )
                osb = opool.tile([128, EMB], BF16, name=f"osb_{c}", tag="osb")
                if _COPY_ENG[c] == "scalar":
                    nc.scalar.copy(out=osb[:], in_=po[:])
                else:
                    nc.vector.tensor_copy(out=osb[:], in_=po[:])

                r0 = c * 128
                for (lo, hi, ring) in _PIECES[c]:
                    eng = {"sync": nc.sync, "scalar": nc.scalar,
                           "gpsimd": nc.gpsimd}[ring]
                    eng.dma_start(
                        out=out[r0 + lo:r0 + hi, :],
                        in_=osb[lo:hi, :],
                    )

    nc.compile()
    return nc


_CACHE: dict = {}


def _get_nc():
    if "nc" not in _CACHE:
        _CACHE["nc"] = build()
    return _CACHE["nc"]


def run(inputs, **spmd_kwargs):
    nc = _get_nc()
    x = np.ascontiguousarray(inputs["x"].reshape(-1)).astype(np.uint16)
    us = [
        np.ascontiguousarray(inputs[f"U{j}"], dtype=np.float32) for j in range(6)
    ]
    aux, aux2 = _aux_table(us)
    in_maps = []
    for i in range(N_CORES):
        in_maps.append(
            {"x": x[i * PER_CORE:(i + 1) * PER_CORE], "aux": aux, "aux2": aux2}
        )
    res = run_bass_kernel_spmd(
        nc, in_maps, core_ids=list(range(N_CORES)), **spmd_kwargs
    )
    shards = [
        np.asarray(res.results[i]["out"]).astype(np.float32)
        for i in range(N_CORES)
    ]
    full = np.concatenate(shards, axis=0).reshape(4, 2048, EMB)
    return full, res


def kernel(**inputs) -> np.ndarray:
    return run(inputs)[0]
